# revision 13
# baseline (speedup 1.0000x reference)
"""Sparse (tanh-clipped, key-masked) dot-product attention on 8 trn2 NeuronCores.

Reference computation (per batch b, head h):
    logits = (Q @ K^T) / 8
    logits = 10 * tanh(logits)
    logits[masked keys] = -inf          (mask is per (batch, key))
    out = softmax(logits) @ V

Strategy (v2 — engine-split softmax):
  - Host: gather K/V down to the unmasked keys (~50% of 2048), pad to a
    multiple of 128.  Pre-transpose Q/K to [64, S] so the contraction dim
    is on partitions.
  - Device (per core: one batch, 8 heads), per 128-key tile:
      PE : S_T[k, q] = Kt-tile^T @ Qt            (fp32r, 1 cycle/col)
      ACT: t = tanh(s/8) -> fp16                 (the only ACT pass)
      DVE: i16 = round(t*A + B)  (tensor_scalar, 4x perf mode)
           -- Schraudolph: the int16 bit pattern IS fp16 of
              2^14 * exp(10*t - 10), up to mantissa-interp wiggle.
           For a (tunable) share of tiles, one custom DVE op multiplies by
           a deg-2 minimax correction of the 2^f vs (1+f) mantissa error;
           uncorrected tiles instead get a half-wiggle bias offset.
    Softmax needs no max-subtraction (10*tanh bounded); the +14 exponent
    shift keeps all int16 codes positive; pad keys get bias -32768 which
    bitcasts to fp16 -0.0 (exact zero weight).
  - Phase 2 (per 128-query block, overlapped with the next head's phase 1):
      PE : out_q[128q, 65] = sum_t P_t[:, q-block]^T @ [V|1]-tile  (fp16)
      DVE: reciprocal of the ones-column;  GPSIMD: scale;  DMA out.
"""

import sys

if "/opt/trn_rl_repo" not in sys.path:
    sys.path.insert(0, "/opt/trn_rl_repo")

import numpy as np

import concourse.tile as tile
from concourse import bacc, mybir
from concourse.bass_utils import run_bass_kernel_spmd

B, H, S, D = 4, 16, 2048, 64
N_CORES = 8
HPC = B * H // N_CORES  # heads per core = 8 (each core: 1 batch, 8 heads)
Q_CHUNK = 512  # PSUM-bank / fp32 moving-operand limit
F32 = mybir.dt.float32
F32R = mybir.dt.float32r
F16 = mybir.dt.float16
I16 = mybir.dt.int16

LOG2E = 1.4426950408889634
SCHR_A = 1024.0 * 10.0 * LOG2E  # 14773.157218702585
SCHR_B = 15360.0 + 1024.0 * 14.0 - SCHR_A  # exponent shift +14 keeps i16 > 0
SCHR_B_UNC = SCHR_B - 1024.0 * 0.0430  # half-wiggle offset, uncorrected tiles
# deg-2 minimax (rel err 3.4e-3) of 2^(m-1)/m on [1,2]
CORR_C2 = 0.23369999579388973
CORR_C1 = -0.6942116428666824
CORR_C0 = 1.4570674748152992
MASK_F = float(np.uint32(0x007FFFFF).view(np.float32))
CORR_OF_9 = 4  # tiles with (h*n_kt+t) % 9 < CORR_OF_9 get the mantissa fix

_kernel_cache = {}
_expfix_op = None


def _get_expfix():
    """Register the EXPFIX custom DVE op (idempotent, process-global)."""
    global _expfix_op
    if _expfix_op is not None:
        return _expfix_op
    from concourse import dve_ops
    from concourse.dve_ops import DveOp
    from concourse.dve_spec import (
        C0,
        C1,
        C2,
        C3,
        AluOp,
        Bin,
        One,
        Spec,
        Src0,
        _has_src1,
        _spill_c3_to_src1,
        lower,
    )
    from concourse.dve_uop import DveOpSpec

    name = "EXPFIX_ANT"
    for o in dve_ops.OPS:
        if o.name == name:
            _expfix_op = o
            return o

    def _ref(in0, in1, s0, s1, imm2):
        x = np.asarray(in0, np.float32)
        bits = x.view(np.uint32)
        m = ((bits & np.uint32(0x007FFFFF)) | np.uint32(0x3F800000)).view(
            np.float32
        )
        c2 = np.asarray(in1, np.float32).reshape(-1, 1)
        corr = (c2 * m + np.float32(s1)) * m + np.float32(s0)
        return (x * corr).astype(np.float32)

    _m = Bin(AluOp.BITWISE_OR, Bin(AluOp.BITWISE_AND, Src0, C2), One)
    spec = Spec(
        body=_spill_c3_to_src1(Src0 * ((C3 * _m + C1) * _m + C0)),
        reference=_ref,
    )
    row = dve_ops._CUSTOM_DVE_ROW_BASE + len(dve_ops.OPS)
    shas = {}
    for ver in ("v3", "v4"):
        tmp = DveOpSpec(
            name=name, opcode=row, uops=lower(spec, ver=ver), rd1_en=_has_src1(spec)
        )
        shas[ver] = tmp.sha(ver)
    op = DveOp(name, spec, subdim=False, uops_sha=shas)
    dve_ops.OPS.append(op)
    dve_ops._SUB_OPCODE_FOR_NAME[name] = row
    dve_ops.CUSTOM_DVE_SPECS[name] = spec
    _expfix_op = op
    return op


def _build_kernel(n_kp: int, reps: int = 1):
    """Build the per-core Bass program for n_kp (padded) kept keys."""
    expfix = _get_expfix()
    n_kt = n_kp // 128
    nc = bacc.Bacc(None)

    qt_p = nc.declare_dram_parameter("qt", [HPC, D, S], F32R, isOutput=False)
    kt_p = nc.declare_dram_parameter("kt", [HPC, D, n_kp], F32R, isOutput=False)
    v_p = nc.declare_dram_parameter("vaug", [HPC, 128, n_kt, D + 1], F16, isOutput=False)
    bias_p = nc.declare_dram_parameter("bias", [128, 2, n_kt], F32, isOutput=False)
    out_p = nc.declare_dram_parameter("out", [HPC, S, D], F32, isOutput=True)

    n_qc = S // Q_CHUNK  # 4 q-chunks of 512
    n_qi = S // 128  # 16 query row-blocks

    with tile.TileContext(nc) as tc:
        with (
            tc.tile_pool(name="consts", bufs=1) as consts,
            tc.tile_pool(name="inq", bufs=2) as inq,
            tc.tile_pool(name="ink", bufs=2) as ink,
            tc.tile_pool(name="inv", bufs=2) as inv,
            tc.tile_pool(name="act", bufs=4) as act_pool,
            tc.tile_pool(name="i16tmp", bufs=4) as tmp_pool,
            # two headfuls of P tiles for full phase-1/phase-2 overlap;
            # capped for unusually low mask density (SBUF budget)
            tc.tile_pool(
                name="probs", bufs=2 * n_kt + 1 if n_kt <= 12 else n_kt + 4
            ) as probs_pool,
            tc.tile_pool(name="outsb", bufs=6) as out_pool,
            tc.tile_pool(name="ps_st", bufs=2, space="PSUM") as ps_st,
            tc.tile_pool(name="ps_oq", bufs=4, space="PSUM") as ps_oq,
        ):
            bias_sb = consts.tile([128, 2, n_kt], F32)
            c2_sb = consts.tile([128, 1], F32)
            nc.vector.memset(c2_sb, CORR_C2)
            # prime the ACT table set (tanh) while first input DMAs fly
            warm = consts.tile([128, 1], F32)
            nc.vector.memset(warm, 0.0)
            nc.scalar.activation(warm, warm, mybir.ActivationFunctionType.Tanh)

            OGRP = 4  # qblocks per packed PSUM accumulator / staging / DMA

            def phase2_group(hh, g, p_tiles, v_tile):
                # 4 qblocks' PV accumulators packed in ONE PSUM bank
                # ([128, 4, 65] = 1040 B); each [*, j, :] slice stays
                # within the bank so matmul outputs are legal.
                oq_ps = ps_oq.tile([128, OGRP, D + 1], F32, tag="oq")
                for j in range(OGRP):
                    qi = g * OGRP + j
                    for t in range(n_kt):
                        nc.tensor.matmul(
                            oq_ps[:, j, :],
                            lhsT=p_tiles[t][
                                :, qi * 128 : (qi + 1) * 128
                            ].bitcast(F16),
                            rhs=v_tile[:, t, :],
                            start=(t == 0),
                            stop=(t == n_kt - 1),
                        )
                recip = out_pool.tile([128, OGRP, 1], F32, tag="recip")
                nc.vector.reciprocal(recip, oq_ps[:, :, D])
                oq = out_pool.tile([128, OGRP, D], F32, tag="out")
                nc.vector.tensor_tensor(
                    out=oq,
                    in0=oq_ps[:, :, 0:D],
                    in1=recip[:].broadcast_to([128, OGRP, D]),
                    op=mybir.AluOpType.mult,
                )
                # DMA issued off the gpsimd queue to keep SP clear.
                # DRAM rows are q = j*128 + p while the staging tile is
                # [p, j, d] — rearrange the DRAM-side AP to match.
                nc.gpsimd.dma_start(
                    out=out_p[
                        hh, g * OGRP * 128 : (g + 1) * OGRP * 128, :
                    ].rearrange("(j p) d -> p j d", j=OGRP),
                    in_=oq,
                )

            n_g = n_qi // OGRP  # 4 phase-2 groups of 4 qblocks
            heads = [h for _ in range(reps) for h in range(HPC)]
            prev = None  # (head, p_tiles, v_tile) pending phase 2
            per_step = -(-n_g // max(1, n_kt - 1))
            for i, h in enumerate(heads):
                qt_sb = inq.tile([D, S], F32R, tag="qt")
                kt_sb = ink.tile([D, n_kp], F32R, tag="kt")
                if i == 0:
                    # fine-grained first loads so the first matmul starts
                    # as early as possible
                    nc.sync.dma_start(out=kt_sb[:, 0:128], in_=kt_p[h][:, 0:128])
                    for qc in range(n_qc):
                        nc.sync.dma_start(
                            out=qt_sb[:, qc * Q_CHUNK : (qc + 1) * Q_CHUNK],
                            in_=qt_p[h][:, qc * Q_CHUNK : (qc + 1) * Q_CHUNK],
                        )
                        if qc == 0:
                            # bias lands after the first q-chunk; needed only
                            # by the first DVE pass (~5 us in)
                            nc.sync.dma_start(out=bias_sb, in_=bias_p[:])
                    if n_kp > 128:
                        nc.sync.dma_start(out=kt_sb[:, 128:], in_=kt_p[h][:, 128:])
                else:
                    nc.sync.dma_start(out=qt_sb, in_=qt_p[h])
                    nc.sync.dma_start(out=kt_sb, in_=kt_p[h])
                v_sb = inv.tile([128, n_kt, D + 1], F16, tag="v")
                nc.sync.dma_start(out=v_sb, in_=v_p[h])

                p_tiles = []
                g_cursor = 0
                for t in range(n_kt):
                    t_sb = act_pool.tile([128, S], F16, tag="tanh")
                    # half-size PSUM logit tiles (2 banks each, double-
                    # buffered) so tile t+1's matmuls overlap tile t's tanh
                    for half in range(2):
                        st_ps = ps_st.tile([128, S // 2], F32, tag="st")
                        for qc in range(n_qc // 2):
                            qa = half * (S // 2) + qc * Q_CHUNK
                            nc.tensor.matmul(
                                st_ps[:, qc * Q_CHUNK : (qc + 1) * Q_CHUNK],
                                lhsT=kt_sb[:, t * 128 : (t + 1) * 128],
                                rhs=qt_sb[:, qa : qa + Q_CHUNK],
                                start=True,
                                stop=True,
                            )
                        nc.scalar.activation(
                            t_sb[:, half * (S // 2) : (half + 1) * (S // 2)],
                            st_ps,
                            mybir.ActivationFunctionType.Tanh,
                            scale=0.125,
                        )
                    p_sb = probs_pool.tile([128, S], I16, tag="p")
                    corrected = ((h * n_kt + t) % 9) < CORR_OF_9
                    if corrected:
                        tmp = tmp_pool.tile([128, S], I16, tag="i16")
                        nc.vector.tensor_scalar(
                            out=tmp,
                            in0=t_sb,
                            scalar1=SCHR_A,
                            scalar2=bias_sb[:, 0, t : t + 1],
                            op0=mybir.AluOpType.mult,
                            op1=mybir.AluOpType.add,
                        )
                        nc.vector._custom_dve(
                            expfix,
                            out=p_sb[:].bitcast(F16),
                            in0=tmp[:].bitcast(F16),
                            in1=c2_sb,
                            s0=CORR_C0,
                            s1=CORR_C1,
                            imm2=MASK_F,
                        )
                    else:
                        nc.vector.tensor_scalar(
                            out=p_sb,
                            in0=t_sb,
                            scalar1=SCHR_A,
                            scalar2=bias_sb[:, 1, t : t + 1],
                            op0=mybir.AluOpType.mult,
                            op1=mybir.AluOpType.add,
                        )
                    p_tiles.append(p_sb)
                    # overlap: drain the previous head's phase 2 under this
                    # head's phase-1 work
                    if prev is not None and t >= 1:
                        for _ in range(min(per_step, n_g - g_cursor)):
                            phase2_group(prev[0], g_cursor, prev[1], prev[2])
                            g_cursor += 1
                if prev is not None:
                    for g in range(g_cursor, n_g):
                        phase2_group(prev[0], g, prev[1], prev[2])
                prev = (h, p_tiles, v_sb)
            for g in range(n_g):
                phase2_group(prev[0], g, prev[1], prev[2])
    if not nc.is_finalized():
        nc.finalize()
    return nc


def _prep_inputs(q, k, v, mask):
    """Host-side shard + gather + layout. Returns (in_maps, n_kp)."""
    keep = [np.flatnonzero(~mask[b, :, 0]) for b in range(B)]
    n_kp = max(128, -(-max(len(kb) for kb in keep) // 128) * 128)
    n_kt = n_kp // 128

    in_maps = []
    for c in range(N_CORES):
        b = c // 2
        h0 = (c % 2) * HPC
        kb = keep[b]
        nk = len(kb)

        qt = np.ascontiguousarray(q[b, h0 : h0 + HPC].transpose(0, 2, 1))

        kg = np.zeros((HPC, n_kp, D), np.float32)
        kg[:, :nk] = k[b, h0 : h0 + HPC][:, kb]
        kt = np.ascontiguousarray(kg.transpose(0, 2, 1))

        vg = np.zeros((HPC, n_kp, D + 1), np.float32)
        vg[:, :nk, :D] = v[b, h0 : h0 + HPC][:, kb]
        vg[:, :, D] = 1.0
        # [HPC, n_kt, 128, 65] -> [HPC, 128, n_kt, 65] (partition-major)
        vaug = np.ascontiguousarray(
            vg.reshape(HPC, n_kt, 128, D + 1).transpose(0, 2, 1, 3)
        ).astype(np.float16)

        # Schraudolph bias columns: [:, 0, t] corrected-class B, [:, 1, t]
        # uncorrected-class B (half-wiggle offset); pad keys -> -32768
        # (int16 saturates; bitcasts to fp16 -0.0 => exact zero weight).
        bias = np.empty((128, 2, n_kt), np.float32)
        bias[:, 0, :] = SCHR_B
        bias[:, 1, :] = SCHR_B_UNC
        idx = np.arange(n_kp).reshape(n_kt, 128).T  # [128, n_kt]
        pad = idx >= nk
        bias[:, 0, :][pad] = -32768.0
        bias[:, 1, :][pad] = -32768.0

        in_maps.append({"qt": qt, "kt": kt, "vaug": vaug, "bias": bias})
    return in_maps, n_kp


def kernel(q, k, v, mask, _trace=False):
    q = np.asarray(q, np.float32)
    k = np.asarray(k, np.float32)
    v = np.asarray(v, np.float32)
    mask = np.asarray(mask, bool)
    assert q.shape == k.shape == v.shape == (B, H, S, D), (q.shape,)
    assert mask.shape == (B, S, 1), (mask.shape,)

    in_maps, n_kp = _prep_inputs(q, k, v, mask)
    if n_kp not in _kernel_cache:
        _kernel_cache[n_kp] = _build_kernel(n_kp)
    nc = _kernel_cache[n_kp]

    # a core occasionally comes up wedged (NRT_EXEC_UNIT_UNRECOVERABLE,
    # self-recovers in ~30 s) — retry rather than fail the whole call
    import time as _time

    res = None
    for attempt in range(3):
        try:
            res = run_bass_kernel_spmd(
                nc, in_maps, list(range(N_CORES)), trace=_trace
            )
            break
        except Exception:
            if attempt == 2:
                raise
            _time.sleep(30)
    out = np.empty((B, H, S, D), np.float32)
    for c in range(N_CORES):
        b = c // 2
        h0 = (c % 2) * HPC
        out[b, h0 : h0 + HPC] = res.results[c]["out"]
    if _trace:
        return out, res
    return out


if __name__ == "__main__":
    rng = np.random.default_rng(0)
    q = rng.standard_normal((B, H, S, D), np.float32)
    k = rng.standard_normal((B, H, S, D), np.float32)
    v = rng.standard_normal((B, H, S, D), np.float32)
    mask = rng.integers(0, 2, (B, S, 1)).astype(bool)
    out = kernel(q, k, v, mask)
    print("out", out.shape, out.dtype, float(np.abs(out).max()))


# revision 16
# speedup vs baseline: 1.3206x; 1.3206x over previous
"""Sparse (tanh-clipped, key-masked) dot-product attention on 8 trn2 NeuronCores.

Reference computation (per batch b, head h):
    logits = (Q @ K^T) / 8
    logits = 10 * tanh(logits)
    logits[masked keys] = -inf          (mask is per (batch, key))
    out = softmax(logits) @ V

Strategy (v2 — engine-split softmax):
  - Host: gather K/V down to the unmasked keys (~50% of 2048), pad to a
    multiple of 128.  Pre-transpose Q/K to [64, S] so the contraction dim
    is on partitions.
  - Device (per core: one batch, 8 heads), per 128-key tile:
      PE : S_T[k, q] = Kt-tile^T @ Qt            (fp32r, 1 cycle/col)
      ACT: t = tanh(s/8) -> fp16                 (the only ACT pass)
      DVE: i16 = round(t*A + B)  (tensor_scalar, 4x perf mode)
           -- Schraudolph: the int16 bit pattern IS fp16 of
              2^14 * exp(10*t - 10), up to mantissa-interp wiggle.
           For a (tunable) share of tiles, one custom DVE op multiplies by
           a deg-2 minimax correction of the 2^f vs (1+f) mantissa error;
           uncorrected tiles instead get a half-wiggle bias offset.
    Softmax needs no max-subtraction (10*tanh bounded); the +14 exponent
    shift keeps all int16 codes positive; pad keys get bias -32768 which
    bitcasts to fp16 -0.0 (exact zero weight).
  - Phase 2 (per 128-query block, overlapped with the next head's phase 1):
      PE : out_q[128q, 65] = sum_t P_t[:, q-block]^T @ [V|1]-tile  (fp16)
      DVE: reciprocal of the ones-column;  GPSIMD: scale;  DMA out.
"""

import sys

if "/opt/trn_rl_repo" not in sys.path:
    sys.path.insert(0, "/opt/trn_rl_repo")

import numpy as np

import concourse.tile as tile
from concourse import bacc, mybir
from concourse.bass_utils import run_bass_kernel_spmd

B, H, S, D = 4, 16, 2048, 64
N_CORES = 8
HPC = B * H // N_CORES  # heads per core = 8 (each core: 1 batch, 8 heads)
Q_CHUNK = 512  # PSUM-bank / fp32 moving-operand limit
F32 = mybir.dt.float32
F32R = mybir.dt.float32r
F16 = mybir.dt.float16
I16 = mybir.dt.int16

LOG2E = 1.4426950408889634
SCHR_A = 1024.0 * 10.0 * LOG2E  # 14773.157218702585
SCHR_B = 15360.0 + 1024.0 * 14.0 - SCHR_A  # exponent shift +14 keeps i16 > 0
SCHR_B_UNC = SCHR_B - 1024.0 * 0.0430  # half-wiggle offset, uncorrected tiles
# deg-2 minimax (rel err 3.4e-3) of 2^(m-1)/m on [1,2]
CORR_C2 = 0.23369999579388973
CORR_C1 = -0.6942116428666824
CORR_C0 = 1.4570674748152992
MASK_F = float(np.uint32(0x007FFFFF).view(np.float32))
CORR_OF_9 = 4  # tiles with (h*n_kt+t) % 9 < CORR_OF_9 get the mantissa fix
QK_F16 = True  # Q/K in fp16: fast FWL weight loads + half the DMA; ~3e-3 logit noise

_kernel_cache = {}
_expfix_op = None
ABLATE = set()  # timing ablation: subset of {'tanh','passa','expfix','pv','norm'}


def _get_expfix():
    """Register the EXPFIX custom DVE op (idempotent, process-global)."""
    global _expfix_op
    if _expfix_op is not None:
        return _expfix_op
    from concourse import dve_ops
    from concourse.dve_ops import DveOp
    from concourse.dve_spec import (
        C0,
        C1,
        C2,
        C3,
        AluOp,
        Bin,
        One,
        Spec,
        Src0,
        _has_src1,
        _spill_c3_to_src1,
        lower,
    )
    from concourse.dve_uop import DveOpSpec

    name = "EXPFIX_ANT"
    for o in dve_ops.OPS:
        if o.name == name:
            _expfix_op = o
            return o

    def _ref(in0, in1, s0, s1, imm2):
        x = np.asarray(in0, np.float32)
        bits = x.view(np.uint32)
        m = ((bits & np.uint32(0x007FFFFF)) | np.uint32(0x3F800000)).view(
            np.float32
        )
        c2 = np.asarray(in1, np.float32).reshape(-1, 1)
        corr = (c2 * m + np.float32(s1)) * m + np.float32(s0)
        return (x * corr).astype(np.float32)

    _m = Bin(AluOp.BITWISE_OR, Bin(AluOp.BITWISE_AND, Src0, C2), One)
    spec = Spec(
        body=_spill_c3_to_src1(Src0 * ((C3 * _m + C1) * _m + C0)),
        reference=_ref,
    )
    row = dve_ops._CUSTOM_DVE_ROW_BASE + len(dve_ops.OPS)
    shas = {}
    for ver in ("v3", "v4"):
        tmp = DveOpSpec(
            name=name, opcode=row, uops=lower(spec, ver=ver), rd1_en=_has_src1(spec)
        )
        shas[ver] = tmp.sha(ver)
    op = DveOp(name, spec, subdim=False, uops_sha=shas)
    dve_ops.OPS.append(op)
    dve_ops._SUB_OPCODE_FOR_NAME[name] = row
    dve_ops.CUSTOM_DVE_SPECS[name] = spec
    _expfix_op = op
    return op


def _build_kernel(n_kp: int, reps: int = 1):
    """Build the per-core Bass program for n_kp (padded) kept keys."""
    expfix = _get_expfix()
    n_kt = n_kp // 128
    nc = bacc.Bacc(None)

    qk_dt = F16 if QK_F16 else F32R
    qt_p = nc.declare_dram_parameter("qt", [HPC, D, S], qk_dt, isOutput=False)
    kt_p = nc.declare_dram_parameter("kt", [HPC, D, n_kp], qk_dt, isOutput=False)
    v_p = nc.declare_dram_parameter("vaug", [HPC, 128, n_kt, D + 1], F16, isOutput=False)
    bias_p = nc.declare_dram_parameter("bias", [128, 2, n_kt], F32, isOutput=False)
    out_p = nc.declare_dram_parameter("out", [HPC, S, D], F32, isOutput=True)

    n_qc = S // Q_CHUNK  # 4 q-chunks of 512
    n_qi = S // 128  # 16 query row-blocks

    with tile.TileContext(nc) as tc:
        with (
            tc.tile_pool(name="consts", bufs=1) as consts,
            tc.tile_pool(name="inq", bufs=2) as inq,
            tc.tile_pool(name="ink", bufs=2) as ink,
            tc.tile_pool(name="inv", bufs=2) as inv,
            tc.tile_pool(name="act", bufs=4) as act_pool,
            tc.tile_pool(name="i16tmp", bufs=4) as tmp_pool,
            # two headfuls of P tiles for full phase-1/phase-2 overlap;
            # capped for unusually low mask density (SBUF budget)
            tc.tile_pool(
                name="probs", bufs=2 * n_kt + 1 if n_kt <= 12 else n_kt + 4
            ) as probs_pool,
            tc.tile_pool(name="outsb", bufs=6) as out_pool,
            tc.tile_pool(name="ps_st", bufs=2, space="PSUM") as ps_st,
            tc.tile_pool(name="ps_oq", bufs=4, space="PSUM") as ps_oq,
        ):
            bias_sb = consts.tile([128, 2, n_kt], F32)
            c2_sb = consts.tile([128, 1], F32)
            nc.vector.memset(c2_sb, CORR_C2)
            # prime the ACT table set (tanh) while first input DMAs fly
            warm = consts.tile([128, 1], F32)
            nc.vector.memset(warm, 0.0)
            nc.scalar.activation(warm, warm, mybir.ActivationFunctionType.Tanh)

            OGRP = 4  # qblocks per packed PSUM accumulator / staging / DMA

            def phase2_group(hh, g, p_tiles, v_tile):
                # 4 qblocks' PV accumulators packed in ONE PSUM bank
                # ([128, 4, 65] = 1040 B); each [*, j, :] slice stays
                # within the bank so matmul outputs are legal.
                oq_ps = ps_oq.tile([128, OGRP, D + 1], F32, tag="oq")
                if "pv" in ABLATE:
                    return
                for j in range(OGRP):
                    qi = g * OGRP + j
                    for t in range(n_kt):
                        nc.tensor.matmul(
                            oq_ps[:, j, :],
                            lhsT=p_tiles[t][
                                :, qi * 128 : (qi + 1) * 128
                            ].bitcast(F16),
                            rhs=v_tile[:, t, :],
                            start=(t == 0),
                            stop=(t == n_kt - 1),
                        )
                if "norm" in ABLATE:
                    return
                recip = out_pool.tile([128, OGRP, 1], F32, tag="recip")
                nc.vector.reciprocal(recip, oq_ps[:, :, D])
                oq = out_pool.tile([128, OGRP, D], F32, tag="out")
                nc.vector.tensor_tensor(
                    out=oq,
                    in0=oq_ps[:, :, 0:D],
                    in1=recip[:].broadcast_to([128, OGRP, D]),
                    op=mybir.AluOpType.mult,
                )
                # DMA issued off the gpsimd queue to keep SP clear.
                # DRAM rows are q = j*128 + p while the staging tile is
                # [p, j, d] — rearrange the DRAM-side AP to match.
                nc.gpsimd.dma_start(
                    out=out_p[
                        hh, g * OGRP * 128 : (g + 1) * OGRP * 128, :
                    ].rearrange("(j p) d -> p j d", j=OGRP),
                    in_=oq,
                )

            n_g = n_qi // OGRP  # 4 phase-2 groups of 4 qblocks
            heads = [h for _ in range(reps) for h in range(HPC)]
            prev = None  # (head, p_tiles, v_tile) pending phase 2
            per_step = -(-n_g // max(1, n_kt - 1))
            for i, h in enumerate(heads):
                qt_sb = inq.tile([D, S], qk_dt, tag="qt")
                kt_sb = ink.tile([D, n_kp], qk_dt, tag="kt")
                if i == 0:
                    # fine-grained first loads so the first matmul starts
                    # as early as possible
                    nc.sync.dma_start(out=kt_sb[:, 0:128], in_=kt_p[h][:, 0:128])
                    for qc in range(n_qc):
                        nc.sync.dma_start(
                            out=qt_sb[:, qc * Q_CHUNK : (qc + 1) * Q_CHUNK],
                            in_=qt_p[h][:, qc * Q_CHUNK : (qc + 1) * Q_CHUNK],
                        )
                        if qc == 0:
                            # bias lands after the first q-chunk; needed only
                            # by the first DVE pass (~5 us in)
                            nc.sync.dma_start(out=bias_sb, in_=bias_p[:])
                    if n_kp > 128:
                        nc.sync.dma_start(out=kt_sb[:, 128:], in_=kt_p[h][:, 128:])
                else:
                    nc.sync.dma_start(out=qt_sb, in_=qt_p[h])
                    nc.sync.dma_start(out=kt_sb, in_=kt_p[h])
                v_sb = inv.tile([128, n_kt, D + 1], F16, tag="v")
                nc.sync.dma_start(out=v_sb, in_=v_p[h])

                p_tiles = []
                g_cursor = 0
                for t in range(n_kt):
                    t_sb = act_pool.tile([128, S], F16, tag="tanh")
                    # half-size PSUM logit tiles (2 banks each, double-
                    # buffered) so tile t+1's matmuls overlap tile t's tanh
                    for half in range(2):
                        st_ps = ps_st.tile([128, S // 2], F32, tag="st")
                        for qc in range(n_qc // 2):
                            qa = half * (S // 2) + qc * Q_CHUNK
                            nc.tensor.matmul(
                                st_ps[:, qc * Q_CHUNK : (qc + 1) * Q_CHUNK],
                                lhsT=kt_sb[:, t * 128 : (t + 1) * 128],
                                rhs=qt_sb[:, qa : qa + Q_CHUNK],
                                start=True,
                                stop=True,
                            )
                        if "tanh" not in ABLATE:
                            nc.scalar.activation(
                                t_sb[:, half * (S // 2) : (half + 1) * (S // 2)],
                                st_ps,
                                mybir.ActivationFunctionType.Tanh,
                                scale=0.125,
                            )
                    p_sb = probs_pool.tile([128, S], I16, tag="p")
                    corrected = ((h * n_kt + t) % 9) < CORR_OF_9
                    if "passa" in ABLATE:
                        pass
                    elif corrected and "expfix" not in ABLATE:
                        tmp = tmp_pool.tile([128, S], I16, tag="i16")
                        nc.vector.tensor_scalar(
                            out=tmp,
                            in0=t_sb,
                            scalar1=SCHR_A,
                            scalar2=bias_sb[:, 0, t : t + 1],
                            op0=mybir.AluOpType.mult,
                            op1=mybir.AluOpType.add,
                        )
                        nc.vector._custom_dve(
                            expfix,
                            out=p_sb[:].bitcast(F16),
                            in0=tmp[:].bitcast(F16),
                            in1=c2_sb,
                            s0=CORR_C0,
                            s1=CORR_C1,
                            imm2=MASK_F,
                        )
                    else:
                        nc.vector.tensor_scalar(
                            out=p_sb,
                            in0=t_sb,
                            scalar1=SCHR_A,
                            scalar2=bias_sb[:, 1, t : t + 1],
                            op0=mybir.AluOpType.mult,
                            op1=mybir.AluOpType.add,
                        )
                    p_tiles.append(p_sb)
                    # overlap: drain the previous head's phase 2 under this
                    # head's phase-1 work
                    if prev is not None and t >= 1:
                        for _ in range(min(per_step, n_g - g_cursor)):
                            phase2_group(prev[0], g_cursor, prev[1], prev[2])
                            g_cursor += 1
                if prev is not None:
                    for g in range(g_cursor, n_g):
                        phase2_group(prev[0], g, prev[1], prev[2])
                prev = (h, p_tiles, v_sb)
            for g in range(n_g):
                phase2_group(prev[0], g, prev[1], prev[2])
    if not nc.is_finalized():
        nc.finalize()
    return nc


def _prep_inputs(q, k, v, mask):
    """Host-side shard + gather + layout. Returns (in_maps, n_kp)."""
    keep = [np.flatnonzero(~mask[b, :, 0]) for b in range(B)]
    n_kp = max(128, -(-max(len(kb) for kb in keep) // 128) * 128)
    n_kt = n_kp // 128

    in_maps = []
    for c in range(N_CORES):
        b = c // 2
        h0 = (c % 2) * HPC
        kb = keep[b]
        nk = len(kb)

        qk_np = np.float16 if QK_F16 else np.float32
        qt = np.ascontiguousarray(q[b, h0 : h0 + HPC].transpose(0, 2, 1)).astype(qk_np)

        kg = np.zeros((HPC, n_kp, D), np.float32)
        kg[:, :nk] = k[b, h0 : h0 + HPC][:, kb]
        kt = np.ascontiguousarray(kg.transpose(0, 2, 1)).astype(qk_np)

        vg = np.zeros((HPC, n_kp, D + 1), np.float32)
        vg[:, :nk, :D] = v[b, h0 : h0 + HPC][:, kb]
        vg[:, :, D] = 1.0
        # [HPC, n_kt, 128, 65] -> [HPC, 128, n_kt, 65] (partition-major)
        vaug = np.ascontiguousarray(
            vg.reshape(HPC, n_kt, 128, D + 1).transpose(0, 2, 1, 3)
        ).astype(np.float16)

        # Schraudolph bias columns: [:, 0, t] corrected-class B, [:, 1, t]
        # uncorrected-class B (half-wiggle offset); pad keys -> -32768
        # (int16 saturates; bitcasts to fp16 -0.0 => exact zero weight).
        bias = np.empty((128, 2, n_kt), np.float32)
        bias[:, 0, :] = SCHR_B
        bias[:, 1, :] = SCHR_B_UNC
        idx = np.arange(n_kp).reshape(n_kt, 128).T  # [128, n_kt]
        pad = idx >= nk
        bias[:, 0, :][pad] = -32768.0
        bias[:, 1, :][pad] = -32768.0

        in_maps.append({"qt": qt, "kt": kt, "vaug": vaug, "bias": bias})
    return in_maps, n_kp


def kernel(q, k, v, mask, _trace=False):
    q = np.asarray(q, np.float32)
    k = np.asarray(k, np.float32)
    v = np.asarray(v, np.float32)
    mask = np.asarray(mask, bool)
    assert q.shape == k.shape == v.shape == (B, H, S, D), (q.shape,)
    assert mask.shape == (B, S, 1), (mask.shape,)

    in_maps, n_kp = _prep_inputs(q, k, v, mask)
    if n_kp not in _kernel_cache:
        _kernel_cache[n_kp] = _build_kernel(n_kp)
    nc = _kernel_cache[n_kp]

    # a core occasionally comes up wedged (NRT_EXEC_UNIT_UNRECOVERABLE,
    # self-recovers in ~30 s) — retry rather than fail the whole call
    import time as _time

    res = None
    for attempt in range(3):
        try:
            res = run_bass_kernel_spmd(
                nc, in_maps, list(range(N_CORES)), trace=_trace
            )
            break
        except Exception:
            if attempt == 2:
                raise
            _time.sleep(30)
    out = np.empty((B, H, S, D), np.float32)
    for c in range(N_CORES):
        b = c // 2
        h0 = (c % 2) * HPC
        out[b, h0 : h0 + HPC] = res.results[c]["out"]
    if _trace:
        return out, res
    return out


if __name__ == "__main__":
    rng = np.random.default_rng(0)
    q = rng.standard_normal((B, H, S, D), np.float32)
    k = rng.standard_normal((B, H, S, D), np.float32)
    v = rng.standard_normal((B, H, S, D), np.float32)
    mask = rng.integers(0, 2, (B, S, 1)).astype(bool)
    out = kernel(q, k, v, mask)
    print("out", out.shape, out.dtype, float(np.abs(out).max()))
